# revision 4
# baseline (speedup 1.0000x reference)
"""Bass/Trainium2 kernel for FLAOperator(mode='gla') CPU-fallback scan.

Reference recurrence (per b, h, d lane, over t = 0..N-1):
    s_t = s_{t-1} + sigmoid(q_t * k_t + g_t) * v_t ;  y_t = s_t
i.e. y = cumsum over N of u, with u = sigmoid(q*k + g) * v  (pure elementwise).

Shapes: q,k,v,g,y all [B=2, H=16, N=4096, D=128] f32.

Strategy (8 NeuronCores, SPMD, no collectives):
  - Shard the 32 independent (b,h) recurrences: 4 per core.
  - Host-side prep: transpose each (b,h) slab to [D, N] and cast to bf16.
    The kernel is HBM-bound (the recurrence is elementwise), so bf16 I/O
    halves the traffic: 16 MiB in + 4 MiB out per core vs 40 MiB for f32.
    bf16 input rounding costs ~0.4% relative error on the cumsum (errors
    and signal both grow as sqrt(t)), well inside the 2e-2 gate.
  - SBUF layout [partition = d, free = n]: every DMA descriptor is a 2 KiB
    contiguous run per partition (full line rate, ~425 GB/s measured).
  - The whole recurrence is ONE DVE tensor_tensor_scan per tile (fp32
    internal state, so the accumulation itself is exact); tiles are
    quarter-sequences [128, 1024] chained via initial=prev[:, -1:].
  - Engine balance per quarter: DVE mul(q*k) + scan; Pool (+g, *v);
    ACT sigmoid. DMA issue: sync (q,k,v in), scalar (g in, y out).
"""

from contextlib import ExitStack

import ml_dtypes
import numpy as np

import concourse.bass as bass
import concourse.tile as tile
from concourse import bacc, mybir
from concourse.bass_utils import run_bass_kernel_spmd

B, H, N, D = 2, 16, 4096, 128
N_CORES = 8
BH = B * H                    # 32 independent recurrences
BH_PER_CORE = BH // N_CORES   # 4
P = 128                       # partitions (= D)
NQ = 4                        # quarter-sequence tiles per (b,h)
Q = N // NQ                   # 1024 columns per tile
F32 = mybir.dt.float32
BF16 = mybir.dt.bfloat16
BF16_NP = ml_dtypes.bfloat16

_PROGRAM = None       # cached compiled Bass program (module-level)
LAST_RESULTS = None   # BassKernelResults of the last run (for test harness)


def _build_program() -> bass.Bass:
    nc = bacc.Bacc("TRN2", debug=False, num_devices=N_CORES)

    q_d = nc.dram_tensor("q", [BH_PER_CORE, D, N], BF16, kind="ExternalInput").ap()
    k_d = nc.dram_tensor("k", [BH_PER_CORE, D, N], BF16, kind="ExternalInput").ap()
    v_d = nc.dram_tensor("v", [BH_PER_CORE, D, N], BF16, kind="ExternalInput").ap()
    g_d = nc.dram_tensor("g", [BH_PER_CORE, D, N], BF16, kind="ExternalInput").ap()
    y_d = nc.dram_tensor("y", [BH_PER_CORE, D, N], BF16, kind="ExternalOutput").ap()

    with tile.TileContext(nc) as tc, ExitStack() as ctx:
        io_pool = ctx.enter_context(tc.tile_pool(name="io", bufs=8))
        a_pool = ctx.enter_context(tc.tile_pool(name="a", bufs=3))
        y_pool = ctx.enter_context(tc.tile_pool(name="y", bufs=3))

        for bh in range(BH_PER_CORE):
            prev_y = None
            for j in range(NQ):
                sl = slice(j * Q, (j + 1) * Q)
                qt = io_pool.tile([P, Q], BF16, tag="q")
                kt = io_pool.tile([P, Q], BF16, tag="k")
                vt = io_pool.tile([P, Q], BF16, tag="v")
                gt = io_pool.tile([P, Q], BF16, tag="g")
                nc.sync.dma_start(out=qt[:], in_=q_d[bh, :, sl])
                nc.sync.dma_start(out=kt[:], in_=k_d[bh, :, sl])
                nc.sync.dma_start(out=vt[:], in_=v_d[bh, :, sl])
                nc.scalar.dma_start(out=gt[:], in_=g_d[bh, :, sl])

                # u = sigmoid(q*k + g) * v  (DVE / Pool / ACT / Pool)
                at = a_pool.tile([P, Q], BF16, tag="a")
                nc.vector.tensor_mul(at[:], qt[:], kt[:])
                nc.gpsimd.tensor_add(at[:], at[:], gt[:])
                nc.scalar.activation(
                    at[:], at[:], mybir.ActivationFunctionType.Sigmoid
                )
                ut = a_pool.tile([P, Q], BF16, tag="u")
                nc.gpsimd.tensor_mul(ut[:], at[:], vt[:])

                # y = cumsum(u) along n: one fp32-state scan per quarter,
                # chained through the previous quarter's last column
                yt = y_pool.tile([P, Q], BF16, tag="y")
                nc.vector.tensor_tensor_scan(
                    out=yt[:], data0=ut[:], data1=ut[:],
                    initial=0.0 if prev_y is None else prev_y[:, Q - 1 : Q],
                    op0=mybir.AluOpType.add, op1=mybir.AluOpType.bypass,
                )
                prev_y = yt

                nc.scalar.dma_start(out=y_d[bh, :, sl], in_=yt[:])

    nc.compile()  # bacc backend: wait legalization, reg alloc, nop fusion
    return nc


def kernel(q: np.ndarray, k: np.ndarray, v: np.ndarray, g: np.ndarray) -> np.ndarray:
    global _PROGRAM, LAST_RESULTS
    if _PROGRAM is None:
        _PROGRAM = _build_program()

    def prep(x):
        # [B, H, N, D] f32 -> [BH, D, N] bf16 (time-major per (b,h,d) lane)
        x = np.asarray(x, dtype=np.float32).reshape(BH, N, D)
        return x.transpose(0, 2, 1).astype(BF16_NP)

    qp, kp, vp, gp = prep(q), prep(k), prep(v), prep(g)
    in_maps = []
    for i in range(N_CORES):
        s = slice(i * BH_PER_CORE, (i + 1) * BH_PER_CORE)
        in_maps.append({"q": qp[s], "k": kp[s], "v": vp[s], "g": gp[s]})

    LAST_RESULTS = run_bass_kernel_spmd(_PROGRAM, in_maps, core_ids=list(range(N_CORES)))
    y = np.concatenate([r["y"] for r in LAST_RESULTS.results], axis=0)  # [BH, D, N]
    return y.transpose(0, 2, 1).astype(np.float32).reshape(B, H, N, D)


# revision 7
# speedup vs baseline: 1.1930x; 1.1930x over previous
"""Bass/Trainium2 kernel for FLAOperator(mode='gla') CPU-fallback scan.

Reference recurrence (per b, h, d lane, over t = 0..N-1):
    s_t = s_{t-1} + sigmoid(q_t * k_t + g_t) * v_t ;  y_t = s_t
i.e. y = cumsum over N of u, with u = sigmoid(q*k + g) * v  (pure elementwise).

Shapes: q,k,v,g,y all [B=2, H=16, N=4096, D=128] f32.

Strategy (8 NeuronCores, SPMD, no collectives):
  - Shard the 32 independent (b,h) recurrences: 4 per core.
  - Host-side prep: transpose each (b,h) slab to [D, N] and cast to bf16.
    The kernel is HBM-bound (the recurrence is elementwise), so bf16 I/O
    halves the traffic: 16 MiB in + 4 MiB out per core vs 40 MiB for f32.
    bf16 input rounding costs ~0.4% relative error on the cumsum (errors
    and signal both grow as sqrt(t)), well inside the 2e-2 gate.
  - SBUF layout [partition = d, free = n]: every DMA descriptor is a 2 KiB
    contiguous run per partition (full line rate, ~425 GB/s measured).
  - The whole recurrence is ONE DVE tensor_tensor_scan per tile (fp32
    internal state, so the accumulation itself is exact); tiles are
    quarter-sequences [128, 1024] chained via initial=prev[:, -1:].
  - Engine balance per quarter: DVE mul(q*k) + scan; Pool (+g, *v);
    ACT sigmoid. DMA issue: sync (q,k,v in), scalar (g in, y out).
"""

from contextlib import ExitStack

import ml_dtypes
import numpy as np

import concourse.bass as bass
import concourse.tile as tile
from concourse import bacc, mybir
from concourse.bass_utils import run_bass_kernel_spmd

B, H, N, D = 2, 16, 4096, 128
N_CORES = 8
BH = B * H                    # 32 independent recurrences
BH_PER_CORE = BH // N_CORES   # 4
P = 128                       # partitions (= D)
NQ = 2                        # half-sequence tiles per (b,h)
Q = N // NQ                   # 2048 columns per tile
F32 = mybir.dt.float32
BF16 = mybir.dt.bfloat16
BF16_NP = ml_dtypes.bfloat16

_PROGRAM = None       # cached compiled Bass program (module-level)
LAST_RESULTS = None   # BassKernelResults of the last run (for test harness)


def _build_program() -> bass.Bass:
    nc = bacc.Bacc("TRN2", debug=False, num_devices=N_CORES)

    q_d = nc.dram_tensor("q", [BH_PER_CORE, D, N], BF16, kind="ExternalInput").ap()
    k_d = nc.dram_tensor("k", [BH_PER_CORE, D, N], BF16, kind="ExternalInput").ap()
    v_d = nc.dram_tensor("v", [BH_PER_CORE, D, N], BF16, kind="ExternalInput").ap()
    g_d = nc.dram_tensor("g", [BH_PER_CORE, D, N], BF16, kind="ExternalInput").ap()
    y_d = nc.dram_tensor("y", [BH_PER_CORE, D, N], BF16, kind="ExternalOutput").ap()

    with tile.TileContext(nc) as tc, ExitStack() as ctx:
        io_pool = ctx.enter_context(tc.tile_pool(name="io", bufs=6))
        a_pool = ctx.enter_context(tc.tile_pool(name="a", bufs=3))
        y_pool = ctx.enter_context(tc.tile_pool(name="y", bufs=3))

        for bh in range(BH_PER_CORE):
            prev_y = None
            for j in range(NQ):
                sl = slice(j * Q, (j + 1) * Q)
                qt = io_pool.tile([P, Q], BF16, tag="q")
                kt = io_pool.tile([P, Q], BF16, tag="k")
                vt = io_pool.tile([P, Q], BF16, tag="v")
                gt = io_pool.tile([P, Q], BF16, tag="g")
                nc.sync.dma_start(out=qt[:], in_=q_d[bh, :, sl])
                nc.sync.dma_start(out=kt[:], in_=k_d[bh, :, sl])
                nc.sync.dma_start(out=vt[:], in_=v_d[bh, :, sl])
                nc.scalar.dma_start(out=gt[:], in_=g_d[bh, :, sl])

                # u = sigmoid(q*k + g) * v  (DVE / Pool / ACT / DVE)
                at = a_pool.tile([P, Q], BF16, tag="a")
                nc.vector.tensor_mul(at[:], qt[:], kt[:])
                bt = a_pool.tile([P, Q], BF16, tag="b")
                nc.gpsimd.tensor_add(bt[:], at[:], gt[:])
                nc.scalar.activation(
                    bt[:], bt[:], mybir.ActivationFunctionType.Sigmoid
                )
                ut = a_pool.tile([P, Q], BF16, tag="u")
                nc.vector.tensor_mul(ut[:], bt[:], vt[:])

                # y = cumsum(u) along n: one fp32-state scan per quarter,
                # chained through the previous quarter's last column
                yt = y_pool.tile([P, Q], BF16, tag="y")
                nc.vector.tensor_tensor_scan(
                    out=yt[:], data0=ut[:], data1=ut[:],
                    initial=0.0 if prev_y is None else prev_y[:, Q - 1 : Q],
                    op0=mybir.AluOpType.add, op1=mybir.AluOpType.bypass,
                )
                prev_y = yt

                nc.scalar.dma_start(out=y_d[bh, :, sl], in_=yt[:])

    nc.compile()  # bacc backend: wait legalization, reg alloc, nop fusion
    return nc


def kernel(q: np.ndarray, k: np.ndarray, v: np.ndarray, g: np.ndarray) -> np.ndarray:
    global _PROGRAM, LAST_RESULTS
    if _PROGRAM is None:
        _PROGRAM = _build_program()

    def prep(x):
        # [B, H, N, D] f32 -> [BH, D, N] bf16 (time-major per (b,h,d) lane)
        x = np.asarray(x, dtype=np.float32).reshape(BH, N, D)
        return x.transpose(0, 2, 1).astype(BF16_NP)

    qp, kp, vp, gp = prep(q), prep(k), prep(v), prep(g)
    in_maps = []
    for i in range(N_CORES):
        s = slice(i * BH_PER_CORE, (i + 1) * BH_PER_CORE)
        in_maps.append({"q": qp[s], "k": kp[s], "v": vp[s], "g": gp[s]})

    LAST_RESULTS = run_bass_kernel_spmd(_PROGRAM, in_maps, core_ids=list(range(N_CORES)))
    y = np.concatenate([r["y"] for r in LAST_RESULTS.results], axis=0)  # [BH, D, N]
    return y.transpose(0, 2, 1).astype(np.float32).reshape(B, H, N, D)
